# revision 39
# baseline (speedup 1.0000x reference)
"""Trainium2 Bass kernel for nn_CustomModel_7378753814838.

Math (reference):
    a = x1.reshape(N,R,F); b = x2.reshape(N,R,F)
    d2[k,n,i,j] = ||a[n,i] - b[n,j] - m_k||^2
    kv = exp(-d2 / (2*sigma_k^2))
    out = sum_k w_k * softmax_j(kv[k])           w = softmax(1/sigma_params^2)

Fast path (used when a single kernel dominates w AND |s*d2| is small):
    With s = -1/(2 sigma^2), u = s*d2 is tiny (|u| < ~0.04 for the staged
    inputs), so both exponentials are replaced by 2nd-order Taylor
    expansions, which the softmax normalization makes exact up to O(u^3):
        kv  = e^u    ~ 1 + u + u^2/2          = ((u+1)^2 + 1)/2
        sim = softmax_j(kv) = e^kv / sum e^kv  (common factors cancel)
        e^kv ~ (scaled) kv^2 + 1 ~ (t1+1)^2 + 4 = t1*(t1+2) + 5,
        where t1 = (u+1)^2.
    Device pipeline per sample n:
        PSUM: d2 = [matmul  lhsT=-2(a-m)^T, rhs=b^T]             (dot)
                 + [rank-1  ra_i x 1] + [rank-1  1 x cb_j]       (stats)
        ACT:  t1 = Square(s*d2 + 1)                (one op per group of 4)
        DVE:  qm5 = (t1+2)*t1, accum-> S_i          (scalar_tensor_tensor)
        DVE:  r = 1/(S + 640)                       (reciprocal_approx_fast)
        DVE/GPSIMD: out = (qm5 + 5) * r_i
    Host stages ats=-2(a-m)^T, bts=b^T (bf16, transposed+converted on CPU),
    and the row stats ra=sum_f (a-m)^2, cb=sum_f b^2.  Output returned as
    bf16 and upconverted host-side.

General path (multi-kernel or large exponent): the previous PSUM-
accumulation kernel (exact exp/exp softmax), kept verbatim as fallback.

Sharding: data-parallel over N across 8 cores (16 samples each).
"""

import numpy as np

N, R, F, K = 128, 128, 128, 4
NCORES = 8
NP = N // NCORES  # samples per core


def _bf16():
    import ml_dtypes

    return ml_dtypes.bfloat16


def _host_params(sigmas, means, sigma_params):
    sig = np.asarray(sigmas, dtype=np.float64)
    mu = np.asarray(means, dtype=np.float64)
    sp = np.asarray(sigma_params, dtype=np.float64)
    logits = 1.0 / (sp * sp)
    e = np.exp(logits - logits.max())
    w = e / e.sum()
    KS = [k for k in range(K) if w[k] > 1e-12]
    SCALE = [-1.0 / (2.0 * sig[k] * sig[k]) for k in range(K)]
    return w, KS, SCALE, mu


# --------------------------------------------------------------------------
# fast path: single dominant kernel, Taylor-quadratic softmax
# --------------------------------------------------------------------------


def _build_nc_fast(s):
    from contextlib import ExitStack

    import concourse.bacc as bacc
    import concourse.tile as tile
    from concourse import mybir

    f32 = mybir.dt.float32
    bf16 = mybir.dt.bfloat16
    f8 = mybir.dt.float8e4
    ALU = mybir.AluOpType
    ACTF = mybir.ActivationFunctionType

    NG = NP // 4  # groups of 4 samples (one PSUM bank each)

    nc = bacc.Bacc(
        "TRN2",
        target_bir_lowering=False,
        debug=False,
        enable_asserts=False,
        num_devices=NCORES,
    )
    # fp8 dot operands: d2 errors are scaled by s ~ 4e-5, so e4m3 noise on the
    # products is invisible at the 2e-2 output tolerance; halves the DMA bytes
    ats_d = nc.dram_tensor("ats", [F, NP, R], f8, kind="ExternalInput").ap()
    bts_d = nc.dram_tensor("bts", [F, NP, R], f8, kind="ExternalInput").ap()
    # stat operands for the per-group contract-8 rank-1 matmul.  Matmul
    # operands must sit at partition base 0/32/64, and narrow-partition DMAs
    # are per-partition-bandwidth bound, so pack per-group [8 x (128 lhsT +
    # 512 rhs)] blocks at two bases (even groups @0, odd @32) in one tensor:
    #   stb[b+q, g2*640 + i]            = ra[4g+q, i] (q<4) ; 1.0 (q>=4)
    #   stb[b+p, g2*640 + 128 + q*128+j] = (p==q)     (p<4) ; (p-4==q)*cb[..]
    # with b = 32*(g%2), g2 = g//2.
    stb_d = nc.dram_tensor(
        "stb", [40, (NG // 2) * (R + 4 * R)], bf16, kind="ExternalInput"
    ).ap()
    # output: y[:, n, 0:128] = t1 (the un-normalized quadratic q-values
    # minus 3), y[:, n, 128] = per-row sum of t1.  The final softmax
    # normalize out=(t1+3)/(ss+3R) is a per-element affine with these
    # device-computed coefficients, applied during the host-side unshard.
    y = nc.dram_tensor("y", [R, NP, R + 1], bf16, kind="ExternalOutput").ap()

    with ExitStack() as ctx:
        tc = ctx.enter_context(tile.TileContext(nc))
        singles = ctx.enter_context(tc.tile_pool(name="singles", bufs=1))
        psG = ctx.enter_context(tc.tile_pool(name="psG", bufs=1, space="PSUM"))

        STB = singles.tile([40, (NG // 2) * (R + 4 * R)], bf16)
        ATS = singles.tile([F, NP, R], f8)
        BTS = singles.tile([F, NP, R], f8)
        # 2 HWDGE queues (SP / ACT) carry the inputs; stats ride the GPSIMD
        # SWDGE queue.  Chunked so compute can start early, but not too
        # finely: each dma_start on a queue pays a ~0.65us re-arm.
        half = R + 4 * R  # cols for groups {0,1}; groups {2,3} in the rest
        h0, h1 = slice(0, 8), slice(8, 16)
        nc.sync.dma_start(ATS[:, h0, :], ats_d[:, h0, :])
        nc.scalar.dma_start(BTS[:, h0, :], bts_d[:, h0, :])
        nc.gpsimd.dma_start(STB[:, 0:half], stb_d[:, 0:half])
        nc.sync.dma_start(ATS[:, h1, :], ats_d[:, h1, :])
        nc.scalar.dma_start(BTS[:, h1, :], bts_d[:, h1, :])
        nc.gpsimd.dma_start(STB[:, half:], stb_d[:, half:])

        # warm the ACT Square table during the DMA wait
        warm = singles.tile([1, 2], f32)
        nc.vector.memset(warm[:], 1.0)
        nc.scalar.activation(warm[0:1, 0:1], warm[0:1, 1:2], ACTF.Square)

        T1 = singles.tile([R, NP, R + 1], bf16)

        # one PSUM tile (bank) per group, separate tiles so Tile tracks the
        # ACT reads per bank and PE never serializes behind ACT.  Dots run
        # first (they need only the input chunks), the contract-8 stat
        # matmul (+ ra_i + cb_j rank-1s) closes each bank.
        pGs = [psG.tile([R, 4, F], f32, tag=f"pG{g}", name=f"pG{g}") for g in range(NG)]

        def dots(g):
            for q in range(4):
                n = 4 * g + q
                nc.tensor.matmul(
                    pGs[g][:, q, :],
                    lhsT=ATS[:, n, :],
                    rhs=BTS[:, n, :],
                    start=(q == 0),
                    stop=False,
                )

        def stat(g, lo=0, hi=4, stop=True):
            pb = 32 * (g % 2)
            co = (g // 2) * (R + 4 * R)
            nc.tensor.matmul(
                pGs[g][:, lo:hi, :],
                lhsT=STB[pb : pb + 8, co : co + R],
                rhs=STB[pb : pb + 8, co + R + lo * R : co + R + hi * R],
                start=False,
                stop=stop,
            )

        def tail(g, lo=0, hi=4):
            sg = slice(4 * g + lo, 4 * g + hi)
            # q-value (scaled):  t = (2s*d2 + 1)^2,  q = t + 3
            # (e^(e^u - 1) ~ 1 + u + u^2 = (u + 1/2)^2 + 3/4 ~ (2u+1)^2 + 3,
            #  common factors cancel in the softmax ratio; bias must be a
            #  registered const AP, so use 1.0)
            nc.scalar.activation(
                T1[:, sg, 0:R],
                pGs[g][:, lo:hi, :],
                ACTF.Square,
                bias=1.0,
                scale=float(2 * s),
            )
            # bf16 row-sum accumulator in col 128: sums are ~450-512 so bf16
            # costs ~2e-3 relative on the softmax denominator (2e-2 budget)
            with nc.allow_low_precision(reason="softmax denom, 2e-2 tolerance"):
                nc.vector.tensor_reduce(
                    T1[:, sg, R : R + 1],
                    T1[:, sg, 0:R],
                    axis=mybir.AxisListType.X,
                    op=ALU.add,
                )
            nc.sync.dma_start(y[:, sg, :], T1[:, sg, :])

        # PE order: h0's dots before any STB-gated stat matmul so a late
        # stats DMA can't stall the dot stream.  Last group is processed in
        # two 2-sample halves to shorten the closing Square+reduce chain.
        dots(0)
        dots(1)
        stat(0)
        tail(0)
        stat(1)
        tail(1)
        dots(2)
        stat(2)
        tail(2)
        dots(3)
        stat(3, 0, 2, stop=False)
        tail(3, 0, 2)
        stat(3, 2, 4, stop=True)
        tail(3, 2, 4)

    nc.compile()
    return nc


def _run_fast(x1, x2, s, m, w0, trace, rk):
    from concourse.bass_utils import run_bass_kernel_spmd

    bf = _bf16()
    a = x1.reshape(N, R, F).astype(np.float32) - np.float32(m)
    b = x2.reshape(N, R, F).astype(np.float32)
    ra = np.square(a).sum(-1)  # [N, R] f32
    cb = np.square(b).sum(-1)

    ubound = 2.0 * abs(s) * (float(ra.max()) + float(cb.max()))
    if ubound >= 0.15:
        return None  # Taylor expansion not safe; caller falls back

    import ml_dtypes

    f8 = ml_dtypes.float8_e4m3
    nc = _get_nc(("fast", float(s)), _build_nc_fast, float(s))

    atsf = np.ascontiguousarray((-2.0 * a).transpose(2, 0, 1)).astype(f8)  # [F,N,R]
    btsf = np.ascontiguousarray(b.transpose(2, 0, 1)).astype(f8)

    NG = NP // 4
    in_maps = []
    for c in range(NCORES):
        sl = slice(c * NP, (c + 1) * NP)
        rac, cbc = ra[sl], cb[sl]  # [NP, R] f32
        stb = np.zeros((40, (NG // 2) * (R + 4 * R)), dtype=np.float32)
        for g in range(NG):
            pb = 32 * (g % 2)
            co = (g // 2) * (R + 4 * R)
            for q in range(4):
                stb[pb + q, co : co + R] = rac[4 * g + q]
                cs = co + R + q * R
                stb[pb + q, cs : cs + R] = 1.0
                stb[pb + 4 + q, cs : cs + R] = cbc[4 * g + q]
            stb[pb + 4 : pb + 8, co : co + R] = 1.0
        in_maps.append(
            {
                "ats": np.ascontiguousarray(atsf[:, sl, :]),
                "bts": np.ascontiguousarray(btsf[:, sl, :]),
                "stb": stb.astype(bf),
            }
        )
    for attempt in range(3):
        res = run_bass_kernel_spmd(
            nc, in_maps, core_ids=list(range(NCORES)), trace=trace, **rk
        )
        # unshard + apply the normalization affine out = (t1 + 3) / (ss + 3R)
        # using the device-computed row sums (shipped as y[:, :, 128])
        yv = np.concatenate([r["y"] for r in res.results], axis=1)  # [R, N, R+1]
        t1 = yv[:, :, 0:R].astype(np.float32)
        ssv = yv[:, :, R].astype(np.float32)
        rcp = np.float32(w0) / (ssv + np.float32(3.0 * R))
        out = (t1 + np.float32(3.0)) * rcp[:, :, None]
        out = np.ascontiguousarray(out.transpose(1, 0, 2))
        # softmax rows sum to w0 exactly; a violation means a corrupted
        # device execution -> rerun
        rowerr = np.abs(out.sum(-1) - np.float32(w0)).max()
        if rowerr < 0.02:
            return out, res
    return out, res


# --------------------------------------------------------------------------
# general path (previous kernel): exact exp/exp softmax, any K
# --------------------------------------------------------------------------


def _build_nc_general(sigmas, means, sigma_params):
    from contextlib import ExitStack

    import concourse.bacc as bacc
    import concourse.tile as tile
    from concourse import mybir

    f32 = mybir.dt.float32
    bf16 = mybir.dt.bfloat16
    ALU = mybir.AluOpType
    ACTF = mybir.ActivationFunctionType

    w, KS, SCALE, mu = _host_params(sigmas, means, sigma_params)

    nc = bacc.Bacc(
        "TRN2",
        target_bir_lowering=False,
        debug=False,
        enable_asserts=False,
        num_devices=NCORES,
    )
    x1 = nc.dram_tensor("x1", [NP, R * F], f32, kind="ExternalInput").ap()
    x2 = nc.dram_tensor("x2", [NP, R * F], f32, kind="ExternalInput").ap()
    y = nc.dram_tensor("y", [NP, R, R], f32, kind="ExternalOutput").ap()

    id_p1_d = nc.inline_tensor(np.eye(R).astype(np.float32), name="id_p1").ap()
    id_m2_d = nc.inline_tensor(
        (np.eye(R) * -2.0).astype(np.float32), name="id_m2"
    ).ap()
    qmat_d = nc.inline_tensor(
        np.full((R, R), 0.25, dtype=_bf16()), name="qmat"
    ).ap()
    omat_d = nc.inline_tensor(np.ones((R, R), dtype=_bf16()), name="omat").ap()

    A_src = x1.rearrange("n (i f) -> i n f", i=R)  # [128, NP, 128]
    B_src = x2.rearrange("n (j f) -> j n f", j=R)
    y_dst = y.rearrange("n i j -> i n j")  # [128, NP, 128]

    NG = NP // 4  # groups of 4 samples

    with ExitStack() as ctx:
        tc = ctx.enter_context(tile.TileContext(nc))
        singles = ctx.enter_context(tc.tile_pool(name="singles", bufs=1))
        bigs = ctx.enter_context(tc.tile_pool(name="bigs", bufs=1))
        kbig = ctx.enter_context(tc.tile_pool(name="kbig", bufs=3))
        trash = ctx.enter_context(tc.tile_pool(name="trash", bufs=6))
        psA = ctx.enter_context(tc.tile_pool(name="psA", bufs=2, space="PSUM"))
        psB = ctx.enter_context(tc.tile_pool(name="psB", bufs=2, space="PSUM"))
        psG = ctx.enter_context(tc.tile_pool(name="psG", bufs=4, space="PSUM"))

        # constants
        id_p1 = singles.tile([R, R], f32)
        nc.sync.dma_start(id_p1[:], id_p1_d)
        id_m2 = singles.tile([R, R], f32)
        nc.sync.dma_start(id_m2[:], id_m2_d)
        qmat = singles.tile([R, R], bf16)
        nc.sync.dma_start(qmat[:], qmat_d)
        omat = singles.tile([R, R], bf16)
        nc.sync.dma_start(omat[:], omat_d)

        # inputs, 4-sample chunks for pipelining
        A = bigs.tile([R, NP, F], f32, tag="A")
        B = bigs.tile([R, NP, F], f32, tag="B")
        h0, h1 = slice(0, 8), slice(8, 16)
        nc.sync.dma_start(A[:, h0, :], A_src[:, h0, :])
        nc.scalar.dma_start(B[:, h0, :], B_src[:, h0, :])
        nc.scalar.dma_start(A[:, h1, :], A_src[:, h1, :])
        nc.sync.dma_start(B[:, h1, :], B_src[:, h1, :])

        BT = bigs.tile([R, NP, F], bf16, tag="BT")
        sqB = bigs.tile([R, NP, F], bf16, tag="sqB")
        ATs = {
            k: kbig.tile([R, NP, F], bf16, tag=f"ATs{k}", name=f"ATs{k}") for k in KS
        }
        sqA = {
            k: kbig.tile([R, NP, F], bf16, tag=f"sqA{k}", name=f"sqA{k}") for k in KS
        }

        OUT = bigs.tile([R, NP, F], f32, tag="OUT")
        for g in range(NG):
            s = slice(4 * g, 4 * g + 4)
            # --- transposes via normal matmul (values used; -2 baked in id_m2)
            pA = psA.tile([R, 4, F], f32, tag="pA")
            pB = psB.tile([R, 4, F], f32, tag="pB")
            for q in range(4):
                nc.tensor.matmul(
                    pA[:, q, :],
                    lhsT=A[:, 4 * g + q, :],
                    rhs=id_m2[:],
                    start=True,
                    stop=True,
                )
                nc.tensor.matmul(
                    pB[:, q, :],
                    lhsT=B[:, 4 * g + q, :],
                    rhs=id_p1[:],
                    start=True,
                    stop=True,
                )
            nc.scalar.copy(BT[:, s, :], pB[:])
            for k in KS:
                # ATs = (-2*A^T) + 2m = -2*(A-m)^T   (bf16)
                nc.vector.tensor_scalar(
                    ATs[k][:, s, :], pA[:], 2.0 * float(mu[k]), None, op0=ALU.add
                )
            # --- squares (GPSIMD, bf16) ---
            nc.gpsimd.tensor_mul(sqB[:, s, :], BT[:, s, :], BT[:, s, :])
            for k in KS:
                nc.gpsimd.tensor_mul(
                    sqA[k][:, s, :], ATs[k][:, s, :], ATs[k][:, s, :]
                )
            # --- d2 in PSUM via matmul accumulation, then the exp/softmax tail
            for ki, k in enumerate(KS):
                sc = float(SCALE[k])
                pG = psG.tile([R, 4, F], f32, tag="pG")
                for q in range(4):
                    n = 4 * g + q
                    # -2dot' ; q==0 clears the whole bank's has_written bits
                    nc.tensor.matmul(
                        pG[:, q, :],
                        lhsT=ATs[k][:, n, :],
                        rhs=BT[:, n, :],
                        start=(q == 0),
                        stop=False,
                    )
                for q in range(4):
                    n = 4 * g + q
                    # + sa'2[i] = 0.25*sum_f sqA  (j-broadcast via 0.25-matrix)
                    nc.tensor.matmul(
                        pG[:, q, :],
                        lhsT=sqA[k][:, n, :],
                        rhs=qmat[:],
                        start=False,
                        stop=False,
                    )
                # + sb2[j] for all 4 samples: lhsT = all-ones matrix
                nc.tensor.matmul(
                    pG[:, :, :],
                    lhsT=omat[:],
                    rhs=sqB[:, s, :],
                    start=False,
                    stop=True,
                )
                KV = kbig.tile([R, 4, F], f32, tag="KV")
                E = kbig.tile([R, 4, F], f32, tag="E")
                subs = [(0, 4)]
                for a, b in subs:
                    sb = slice(a, b)
                    nc.scalar.activation(
                        KV[:, sb, :], pG[:, sb, :], ACTF.Exp, scale=sc
                    )
                    nc.scalar.activation(E[:, sb, :], KV[:, sb, :], ACTF.Exp)
                    scol = trash.tile([R, 4], f32, tag="scol")
                    nc.vector.tensor_reduce(
                        scol[:, sb],
                        E[:, sb, :],
                        axis=mybir.AxisListType.X,
                        op=ALU.add,
                    )
                    qcol = trash.tile([R, 4], f32, tag="qcol")
                    nc.vector.reciprocal_approx_fast(qcol[:, sb], scol[:, sb])
                    if w[k] != 1.0:
                        nc.vector.tensor_scalar(
                            qcol[:, sb], qcol[:, sb], float(w[k]), None, op0=ALU.mult
                        )
                    for q in range(a, b):
                        n = 4 * g + q
                        if ki == 0:
                            nc.vector.tensor_scalar(
                                OUT[:, n, :],
                                E[:, q, :],
                                qcol[:, q : q + 1],
                                None,
                                op0=ALU.mult,
                            )
                        else:
                            nc.vector.scalar_tensor_tensor(
                                OUT[:, n, :],
                                E[:, q, :],
                                qcol[:, q : q + 1],
                                OUT[:, n, :],
                                op0=ALU.mult,
                                op1=ALU.add,
                            )
            nc.scalar.dma_start(y_dst[:, s, :], OUT[:, s, :])

    nc.compile()
    return nc


def _run_general(x1, x2, sigmas, means, sigma_params, trace, rk):
    from concourse.bass_utils import run_bass_kernel_spmd

    key = ("gen", sigmas.tobytes(), means.tobytes(), sigma_params.tobytes())
    nc = _get_nc(key, _build_nc_general, sigmas, means, sigma_params)

    in_maps = []
    for c in range(NCORES):
        s = slice(c * NP, (c + 1) * NP)
        in_maps.append({"x1": x1[s], "x2": x2[s]})
    res = run_bass_kernel_spmd(
        nc, in_maps, core_ids=list(range(NCORES)), trace=trace, **rk
    )
    out = np.concatenate([r["y"] for r in res.results], axis=0)
    return out, res


_CACHE = {}


def _get_nc(key, builder, *args):
    if key not in _CACHE:
        _CACHE[key] = builder(*args)
    return _CACHE[key]


def run(x1, x2, sigmas, means, sigma_params, trace=False, **rk):
    x1 = np.ascontiguousarray(x1, dtype=np.float32)
    x2 = np.ascontiguousarray(x2, dtype=np.float32)

    w, KS, SCALE, mu = _host_params(sigmas, means, sigma_params)
    if len(KS) == 1:
        k0 = KS[0]
        r = _run_fast(
            x1, x2, float(SCALE[k0]), float(mu[k0]), float(w[k0]), trace, rk
        )
        if r is not None:
            return r
    return _run_general(x1, x2, sigmas, means, sigma_params, trace, rk)


def kernel(x1, x2, sigmas, means, sigma_params):
    out, _ = run(x1, x2, sigmas, means, sigma_params, trace=False)
    return out
